# revision 22
# baseline (speedup 1.0000x reference)
"""Neural ODE (RK4, 20 steps) Bass kernel for Trainium2, 8 NeuronCores.

Strategy
--------
Data-parallel over batch: each of the 8 cores gets 1024 rows of the
8192-row batch; the small MLP weights are replicated.

On-core layout is *transposed*: the state lives as zT[hidden, batch]
with hidden (512) split into 4 partition chunks of 128, batch (1024)
along the free dimension.  In this layout

    yT = (z @ W)^T = W^T @ zT

maps directly onto the PE's `out = lhsT.T @ rhs` with lhsT = W stored
in its natural [K, M] layout - no transposes anywhere.  Biases become
per-partition scalars (free ACT/stt operands), and the per-sample dt
becomes a free-dim vector that is pre-replicated across partitions on
the host.

All matmul operands use float32r (fp32 storage, PE rounds internally):
full 1 column/cycle PE rate (fp32 proper is 4 cycles/column) at
~1.5e-4 relative error per matmul - measured on hardware.

RK4 with the dt-scaled slope trick: for each eval, one fused
scalar_tensor_tensor computes  t = (psum + b2) * dtv  straight out of
PSUM (dtv = dt/2 or dt), so

    u_next = z + t        (one tensor_tensor, output rounded to f32r)
    s      = t1 + 2*t2 + t3 + t4   (accumulated with stt/adds)
    z'     = z + s/3      (one stt with immediate 1/3)

The two batch halves (512 columns each, one PSUM bank per matmul) are
software-pipelined A/B: while the PE runs half B's matmuls, the DVE
computes half A's t/u updates, so the PE never waits on the elementwise
chain.  All elementwise work stays on the DVE: GPSIMD shares its SBUF
port with the DVE, and offloading ops there measured ~15us/step slower.
"""

import numpy as np
from contextlib import ExitStack

import concourse.bass as bass
import concourse.mybir as mybir
import concourse.tile as tile
from concourse import bacc
from concourse.bass_utils import run_bass_kernel_spmd

P = 128          # partitions
HC = 4           # hidden chunks (512 = 4 * 128)
H = P * HC
B = 1024         # batch per core
NH = 2           # batch halves (PSUM bank = 512 fp32)
BH = B // NH
NCORES = 8
NSTEPS = 20
BATCH = 8192

F32 = mybir.dt.float32
F32R = mybir.dt.float32r
ADD = mybir.AluOpType.add
MULT = mybir.AluOpType.mult
TANH = mybir.ActivationFunctionType.Tanh


def build_nc(nsteps=NSTEPS, l2_m_outer=False, phase_ab=False, skeleton=None,
             no_gpsimd=True, e2_gpsimd=False, skip_s=False, defer_s=False,
             split_psum=False, early_z=False, l1_k_outer_b=False,
             tanh_cols=None):
    """skeleton: None=full kernel, 'pe'=matmuls only, 'peact'=matmuls+ACT.
    no_gpsimd: route the gpsimd ops to DVE instead (SBUF-port contention test).
    """
    nc = bacc.Bacc()

    zt_in = nc.dram_tensor("zt", [P, HC, B], F32, kind="ExternalInput")
    w1_in = nc.dram_tensor("w1", [P, HC, H], F32, kind="ExternalInput")
    w2_in = nc.dram_tensor("w2", [P, HC, H], F32, kind="ExternalInput")
    bias_in = nc.dram_tensor("bias", [P, 2, HC], F32, kind="ExternalInput")
    dts_in = nc.dram_tensor("dts", [P, 2, B], F32, kind="ExternalInput")
    out = nc.dram_tensor("out", [P, HC, B], F32, kind="ExternalOutput")

    with ExitStack() as ctx:
        tc = ctx.enter_context(tile.TileContext(nc))
        pool = ctx.enter_context(tc.tile_pool(name="state", bufs=1))
        psum = ctx.enter_context(
            tc.tile_pool(name="psum", bufs=(4 if split_psum else 8), space="PSUM"))
        if split_psum:
            psum2pool = ctx.enter_context(
                tc.tile_pool(name="psum2", bufs=4, space="PSUM"))
        else:
            psum2pool = psum

        # persistent state
        z = pool.tile([P, HC, B], F32)       # fp32 state
        z_r = pool.tile([P, HC, B], F32R)    # rounded state (eval-1 rhs)
        u = pool.tile([P, HC, B], F32R)      # intermediate RK4 state (rhs)
        h1 = pool.tile([P, HC, B], F32R)     # tanh layer output (rhs)
        t = pool.tile([P, HC, B], F32)       # dt-scaled slope
        t2 = pool.tile([P, HC, B], F32, name="t2") if defer_s else None
        s = pool.tile([P, HC, B], F32)       # RK4 slope accumulator
        w1r = pool.tile([P, HC, H], F32R)
        w2r = pool.tile([P, HC, H], F32R)
        dts = pool.tile([P, 2, B], F32)      # [:,0]=dt/2  [:,1]=dt
        biases = pool.tile([P, 2, HC], F32)  # [:,0,m]=b1  [:,1,m]=b2

        if skeleton is not None:
            # skeleton timing modes read u/h1 without ever writing them
            nc.vector.memset(u[:].bitcast(F32), 0.0)
            nc.vector.memset(h1[:].bitcast(F32), 0.0)
            nc.vector.memset(z_r[:].bitcast(F32), 0.0)

        # preload the tanh ACT table set during the input DMA window
        # (first real tanh otherwise eats the ~2.7us table load)
        warm = pool.tile([P, 8], F32)
        nc.vector.memset(warm[:], 0.0)
        nc.scalar.activation(warm[:], warm[:], TANH)

        # load + funnel everything through the vector engine so downstream
        # consumers carry a single vector-clock wait
        w1_dma = pool.tile([P, HC, H], F32)
        w2_dma = pool.tile([P, HC, H], F32)
        dts_dma = pool.tile([P, 2, B], F32)
        bias_dma = pool.tile([P, 2, HC], F32)
        nc.sync.dma_start(z[:], zt_in[:])
        nc.sync.dma_start(w1_dma[:], w1_in[:])
        nc.sync.dma_start(w2_dma[:], w2_in[:])
        nc.sync.dma_start(dts_dma[:], dts_in[:])
        nc.sync.dma_start(bias_dma[:], bias_in[:])
        nc.vector.tensor_copy(out=z_r[:], in_=z[:])
        nc.vector.tensor_copy(out=w1r[:], in_=w1_dma[:])
        nc.vector.tensor_copy(out=w2r[:], in_=w2_dma[:])
        nc.vector.tensor_copy(out=dts[:], in_=dts_dma[:])
        nc.vector.tensor_copy(out=biases[:], in_=bias_dma[:])

        b1 = lambda m: biases[:, 0, m:m + 1]
        b2 = lambda m: biases[:, 1, m:m + 1]

        deferred = []

        def flush_deferred():
            for fn in deferred:
                fn()
            deferred.clear()

        for step in range(nsteps):
            last_step = step == nsteps - 1
            for e in range(4):
                flush_deferred()
                rhs = z_r if e == 0 else u
                # dt variant: full dt for eval 3 (u4 = z + dt*k3), dt/2 else
                dti = 1 if e == 2 else 0
                def emit_l1(h):
                    cs = slice(h * BH, (h + 1) * BH)
                    # layer 1: psum = W1.T @ rhs ; h1 = tanh(psum + b1)
                    if l1_k_outer_b and h == 1:
                        # k-outer: consume rhs chunks as the DVE finishes them
                        # (half B's u/z_r chunks land just-in-time)
                        ps1s = [psum.tile([P, BH], F32, name="ps1", tag="ps")
                                for _ in range(HC)]
                        for k in range(HC):
                            for m in range(HC):
                                nc.tensor.matmul(
                                    ps1s[m][:],
                                    w1r[:, k, m * P:(m + 1) * P],
                                    rhs[:, k, cs],
                                    start=(k == 0),
                                    stop=(k == HC - 1),
                                )
                        if skeleton != "pe":
                            for m in range(HC):
                                nc.scalar.activation(h1[:, m, cs], ps1s[m][:],
                                                     TANH, bias=b1(m))
                        return
                    for m in range(HC):
                        ps1 = psum.tile([P, BH], F32, name="ps1", tag="ps")
                        for k in range(HC):
                            nc.tensor.matmul(
                                ps1[:],
                                w1r[:, k, m * P:(m + 1) * P],
                                rhs[:, k, cs],
                                start=(k == 0),
                                stop=(k == HC - 1),
                            )
                        if skeleton != "pe":
                            if tanh_cols:  # timing probe: fewer ACT columns
                                nc.scalar.activation(
                                    h1[:, m, h * BH:h * BH + tanh_cols],
                                    ps1[:, :tanh_cols], TANH, bias=b1(m))
                            else:
                                nc.scalar.activation(h1[:, m, cs], ps1[:], TANH, bias=b1(m))

                def emit_update(m, cs, ps2m):
                    if skeleton == "pe":
                        return
                    if skeleton == "peact":
                        # drain psum2 on ACT, skip the whole DVE/gpsimd chain
                        nc.scalar.copy(t[:, m, cs], ps2m[:])
                        return
                    dtv = dts[:, dti, cs]
                    # per-eval slope buffer: e0 writes s directly; with
                    # defer_s, e2 uses t2 so deferred reads of e1's t survive
                    tdst = s if e == 0 else (t2 if (defer_s and e == 2) else t)
                    # t = (psum + b2) * dtv   (eval 0 writes directly to s)
                    nc.vector.scalar_tensor_tensor(
                        out=tdst[:, m, cs], in0=ps2m[:], scalar=b2(m),
                        in1=dtv, op0=ADD, op1=MULT)
                    if e < 3:
                        # u = z + t  (rounded to f32r on write)
                        nc.vector.tensor_add(
                            out=u[:, m, cs], in0=z[:, m, cs], in1=tdst[:, m, cs])
                    if skip_s:
                        return
                    if e == 1:
                        # s += 2*t2 (stt only exists on DVE)
                        def op_e1(m=m, cs=cs):
                            nc.vector.scalar_tensor_tensor(
                                out=s[:, m, cs], in0=t[:, m, cs], scalar=2.0,
                                in1=s[:, m, cs], op0=MULT, op1=ADD)
                        deferred.append(op_e1) if defer_s else op_e1()
                    elif e == 2:
                        # s += t3
                        def op_e2(m=m, cs=cs, tsrc=tdst):
                            eng = nc.gpsimd if e2_gpsimd else (nc.vector if no_gpsimd else nc.gpsimd)
                            eng.tensor_add(
                                out=s[:, m, cs], in0=tsrc[:, m, cs], in1=s[:, m, cs])
                        deferred.append(op_e2) if defer_s else op_e2()
                        if early_z:
                            # z += s/3 with s = t1+2*t2+t3 - runs while eval 3's
                            # matmuls are still in flight, off the critical path
                            nc.vector.scalar_tensor_tensor(
                                out=z[:, m, cs], in0=s[:, m, cs], scalar=1.0 / 3.0,
                                in1=z[:, m, cs], op0=MULT, op1=ADD)
                    elif e == 3:
                        if early_z:
                            # z += t4/3: only one DVE op between the last psum
                            # and the finished state chunk
                            nc.vector.scalar_tensor_tensor(
                                out=z[:, m, cs], in0=t[:, m, cs], scalar=1.0 / 3.0,
                                in1=z[:, m, cs], op0=MULT, op1=ADD)
                        else:
                            eng = nc.vector if no_gpsimd else nc.gpsimd
                            eng.tensor_add(
                                out=s[:, m, cs], in0=t[:, m, cs], in1=s[:, m, cs])
                            # z += s/3
                            nc.vector.scalar_tensor_tensor(
                                out=z[:, m, cs], in0=s[:, m, cs], scalar=1.0 / 3.0,
                                in1=z[:, m, cs], op0=MULT, op1=ADD)
                        if not last_step:
                            # rounded copy for next step's eval-1 rhs
                            # (1-input op; gpsimd runs at line rate, freeing
                            # DVE for the critical t/u chain)
                            eng = nc.vector if no_gpsimd else nc.gpsimd
                            eng.tensor_copy(
                                out=z_r[:, m, cs], in_=z[:, m, cs])
                        else:
                            # stream the finished chunk out while the
                            # remaining chunks still compute
                            nc.sync.dma_start(out[:, m, cs], z[:, m, cs])

                def emit_l2(h):
                    cs = slice(h * BH, (h + 1) * BH)
                    if l2_m_outer:
                        for m in range(HC):
                            ps2m = psum2pool.tile([P, BH], F32, name="ps2",
                                                  tag="ps2" if split_psum else "ps")
                            for k in range(HC):
                                nc.tensor.matmul(
                                    ps2m[:],
                                    w2r[:, k, m * P:(m + 1) * P],
                                    h1[:, k, cs],
                                    start=(k == 0),
                                    stop=(k == HC - 1),
                                )
                            emit_update(m, cs, ps2m)
                    else:
                        # k-outer so L2 can start as h1 chunks land
                        ps2 = [psum2pool.tile([P, BH], F32, name="ps2",
                                              tag="ps2" if split_psum else "ps")
                               for _ in range(HC)]
                        for k in range(HC):
                            for m in range(HC):
                                nc.tensor.matmul(
                                    ps2[m][:],
                                    w2r[:, k, m * P:(m + 1) * P],
                                    h1[:, k, cs],
                                    start=(k == 0),
                                    stop=(k == HC - 1),
                                )
                        for m in range(HC):
                            emit_update(m, cs, ps2[m])

                if phase_ab:
                    emit_l1(0); emit_l1(1); emit_l2(0); emit_l2(1)
                else:
                    emit_l1(0); emit_l2(0); emit_l1(1); emit_l2(1)
    nc.finalize()
    return nc


def _prep_inputs(z_init, delta_t, W1, b1, W2, b2):
    """Full inputs -> list of per-core input dicts (device layout)."""
    w1d = np.ascontiguousarray(W1.reshape(HC, P, H).transpose(1, 0, 2))
    w2d = np.ascontiguousarray(W2.reshape(HC, P, H).transpose(1, 0, 2))
    bias = np.broadcast_to(
        np.stack([b1.reshape(HC, P).T, b2.reshape(HC, P).T], axis=1), (P, 2, HC))
    bias = np.ascontiguousarray(bias)

    in_maps = []
    for c in range(NCORES):
        zc = z_init[c * B:(c + 1) * B]                       # [B, H]
        ztc = np.ascontiguousarray(
            zc.T.reshape(HC, P, B).transpose(1, 0, 2))       # [P, HC, B]
        dtc = delta_t[c * B:(c + 1) * B] * (1.0 / NSTEPS)    # per-step dt
        dts = np.ascontiguousarray(np.broadcast_to(
            np.stack([dtc * 0.5, dtc], axis=0), (P, 2, B))).astype(np.float32)
        in_maps.append({
            "zt": ztc, "w1": w1d, "w2": w2d, "bias": bias.astype(np.float32),
            "dts": dts,
        })
    return in_maps


_NC_CACHE = {}
_EXEC_CACHE = {}
_SHARDED_CACHE = {}
_IN_NAMES_CACHE = {}
_ZEROS_CACHE = {}


def _cached_runner(nsteps):
    """Build nc + a persistently-jitted sharded executable (compile once,
    execute many) - run_bass_via_pjrt re-jits on every call, which hides
    the device time under recompile/reload overhead."""
    if nsteps in _EXEC_CACHE:
        return _EXEC_CACHE[nsteps]
    import jax
    from jax.sharding import Mesh, PartitionSpec
    from jax.experimental.shard_map import shard_map
    from concourse import bass2jax, mybir as _mybir

    nc = build_nc(nsteps)
    bass2jax.install_neuronx_cc_hook()
    partition_name = nc.partition_id_tensor.name if nc.partition_id_tensor else None
    in_names, out_names, out_avals, zero_outs = [], [], [], []
    for alloc in nc.m.functions[0].allocations:
        if not isinstance(alloc, _mybir.MemoryLocationSet):
            continue
        name = alloc.memorylocations[0].name
        if alloc.kind == "ExternalInput":
            if name != partition_name:
                in_names.append(name)
        elif alloc.kind == "ExternalOutput":
            shape = tuple(alloc.tensor_shape)
            dtype = _mybir.dt.np(alloc.dtype)
            out_names.append(name)
            out_avals.append(jax.core.ShapedArray(shape, dtype))
            zero_outs.append(np.zeros(shape, dtype))
    n_params = len(in_names)
    n_outs = len(out_avals)
    all_in_names = list(in_names) + list(out_names)
    if partition_name is not None:
        all_in_names.append(partition_name)

    def _body(*args):
        operands = list(args)
        if partition_name is not None:
            operands.append(bass2jax.partition_id_tensor())
        return tuple(bass2jax._bass_exec_p.bind(
            *operands,
            out_avals=tuple(out_avals),
            in_names=tuple(all_in_names),
            out_names=tuple(out_names),
            lowering_input_output_aliases=(),
            sim_require_finite=True,
            sim_require_nnan=True,
            nc=nc,
        ))

    devices = jax.devices()[:NCORES]
    mesh = Mesh(np.asarray(devices), ("core",))
    in_specs = (PartitionSpec("core"),) * (n_params + n_outs)
    out_specs = (PartitionSpec("core"),) * n_outs
    sharded = jax.jit(
        shard_map(_body, mesh=mesh, in_specs=in_specs, out_specs=out_specs,
                  check_rep=False),
        keep_unused=True,
    )
    _SHARDED_CACHE[nsteps] = sharded
    _IN_NAMES_CACHE[nsteps] = in_names
    _ZEROS_CACHE[nsteps] = zero_outs

    def run(in_maps):
        concat_in = [
            np.concatenate([np.asarray(in_maps[c][name]) for c in range(NCORES)],
                           axis=0)
            for name in in_names
        ]
        concat_zeros = [
            np.zeros((NCORES * z.shape[0], *z.shape[1:]), z.dtype)
            for z in zero_outs
        ]
        out_arrs = sharded(*concat_in, *concat_zeros)
        out_arrs = [np.asarray(o) for o in out_arrs]
        return [
            {name: out_arrs[i].reshape(NCORES, *out_avals[i].shape)[c]
             for i, name in enumerate(out_names)}
            for c in range(NCORES)
        ]

    _EXEC_CACHE[nsteps] = run
    return run


class _Res:
    def __init__(self, results):
        self.results = results
        self.exec_time_ns = None


def _run_cached(inputs, nsteps=NSTEPS):
    run = _cached_runner(nsteps)
    in_maps = _prep_inputs(
        np.asarray(inputs["z_init"], dtype=np.float32),
        np.asarray(inputs["delta_t"], dtype=np.float32),
        np.asarray(inputs["W1"], dtype=np.float32),
        np.asarray(inputs["b1"], dtype=np.float32),
        np.asarray(inputs["W2"], dtype=np.float32),
        np.asarray(inputs["b2"], dtype=np.float32),
    )
    results = run(in_maps)
    outs = []
    for c in range(NCORES):
        o = results[c]["out"]
        outs.append(o.transpose(1, 0, 2).reshape(H, B).T)
    full = np.concatenate(outs, axis=0).astype(np.float32)
    return full, _Res(results)


def _run(inputs, trace=False, nsteps=NSTEPS, **build_kwargs):
    key = (nsteps, tuple(sorted(build_kwargs.items())))
    if key not in _NC_CACHE:
        _NC_CACHE[key] = build_nc(nsteps, **build_kwargs)
    nc = _NC_CACHE[key]
    in_maps = _prep_inputs(
        np.asarray(inputs["z_init"], dtype=np.float32),
        np.asarray(inputs["delta_t"], dtype=np.float32),
        np.asarray(inputs["W1"], dtype=np.float32),
        np.asarray(inputs["b1"], dtype=np.float32),
        np.asarray(inputs["W2"], dtype=np.float32),
        np.asarray(inputs["b2"], dtype=np.float32),
    )
    res = run_bass_kernel_spmd(nc, in_maps, core_ids=list(range(NCORES)),
                               trace=trace)
    outs = []
    for c in range(NCORES):
        o = res.results[c]["out"]                            # [P, HC, B]
        outs.append(o.transpose(1, 0, 2).reshape(H, B).T)    # [B, H]
    full = np.concatenate(outs, axis=0).astype(np.float32)
    return full, res


def kernel(z_init, delta_t, W1, b1, W2, b2):
    full, _ = _run({"z_init": z_init, "delta_t": delta_t, "W1": W1,
                    "b1": b1, "W2": W2, "b2": b2})
    return full
